# revision 17
# baseline (speedup 1.0000x reference)
"""Dcls2_1d (dilated conv with learnable row spacings) on 8 trn2 NeuronCores.

Strategy: data-parallel over batch (16 -> 2 images/core). Host constructs the
dense (O, I, 7, 3) scattered kernel (exact port of the reference bilinear
scatter) and F(4,3)-transforms it over the width taps; each core runs the conv
as an implicit GEMM contracting over C_in=128 (the partition dim).

Winograd F(4,3) over width: 6 multiply-points per 4 output columns instead of
12, cutting the PE's streamed matmul columns to half of the direct method
(172k cols/core -> ~72us streaming floor at 1 col/cycle/2.4GHz). The 7 height
taps stay direct, accumulated in PSUM per 32-row strip (512-col PSUM banks).

DVE throughput hygiene (TRN2 tensor_tensor only reaches 2x mode for 16-bit
step-1 4B-aligned operands):
 - host pre-splits the padded image into 6 width-phase planes
   (cols 4q+k, k=0..5) so every input-transform read is contiguous fp16;
 - the output transform writes 4 per-phase blocks (contiguous) instead of
   interleaving; the host de-interleaves after the run;
 - m-points are evacuated PSUM->SBUF as fp16 by the scalar engine (bias for
   the whole A^T rides on m1, whose output coefficients are all ones).

Outputs are DMA'd as fp16 (halves store traffic); the host converts to f32.
Measured rel err (max-abs / max|expected|): ~4.5e-3 vs the fp32 reference.

Input DMAs are priority-ordered (first transform block's planes + first tap
weights first), issued from three engine queues in parallel. A burst of
dummy matmuls warms the PE clock gate (HAM) while inputs are in flight.
"""
import os
import sys
import time

sys.path.insert(0, "/opt/trn_rl_repo")

import numpy as np

import concourse.bass as bass
import concourse.tile as tile
from concourse import bacc, mybir
from concourse import bass_utils

# ---- problem constants (hardcoded per contract) ----
K_H, K_W = 3, 3
LIM = 2            # DIL // 2
KH_EFF = 7         # K_H + 2 * LIM
PAD_H, PAD_W = 3, 1
B, CIN, H, W = 16, 128, 64, 64
COUT = 256
N_CORES = 8
BPC = B // N_CORES                  # images per core
HP, WP = H + 2 * PAD_H, W + 2 * PAD_W   # 70, 66
NPIX = H * W                        # 4096
OH = COUT // 128                    # 2 halves of out channels

NJ = 6                              # F(4,3) points
NQ = W // 4                         # output quads per row: 16
NPLANES = 6                         # width-phase input planes
PLANE = HP * NQ                     # cols per plane: 1120
KCOLS = OH * NJ * KH_EFF * 128      # 10752
RB = [(0, 38), (38, 70)]            # input-transform row blocks (img1)
RB0 = [(0, 38), (38, 64), (64, 70)]  # img0: split so strip 1 unblocks early
STRIPS_STD = [(0, 32), (32, 64)]
STRIPS_LAST = [(0, 32), (32, 48), (48, 64)]   # finer tail on the last block

WARMUP = int(os.environ.get("DCLS_WARMUP", "40"))

MMDT = mybir.dt.float16
NPDT = np.float16

_NC_CACHE = None
_last_in_maps = None  # stashed for test.py's profiled re-run

# F(4,3) weight transform (correlation convention, points 0,+-1,+-2,inf)
G_F43 = np.array([
    [1 / 4, 0, 0],
    [-1 / 6, -1 / 6, -1 / 6],
    [-1 / 6, 1 / 6, -1 / 6],
    [1 / 24, 1 / 12, 1 / 6],
    [1 / 24, -1 / 12, 1 / 6],
    [0, 0, 1],
], np.float32)


def _build_kernel_np(weight: np.ndarray, P1: np.ndarray) -> np.ndarray:
    """Exact numpy port of reference.build_kernel (fp32)."""
    weight = weight.astype(np.float32, copy=False)
    kh = np.arange(K_H, dtype=np.float32)[None, None, :, None]
    pos = kh + LIM + np.clip(P1.astype(np.float32, copy=False), -LIM, LIM)
    p0 = np.floor(pos)
    frac = pos - p0
    p0i = p0.astype(np.int32)
    rng = np.arange(KH_EFF, dtype=np.int32)
    oh0 = (p0i[..., None] == rng).astype(np.float32)
    oh1 = ((p0i + 1)[..., None] == rng).astype(np.float32)
    return (
        np.einsum("oihw,oihwk->oikw", weight * (1.0 - frac), oh0)
        + np.einsum("oihw,oihwk->oikw", weight * frac, oh1)
    ).astype(np.float32)


def _splits(total, n):
    """n near-equal [lo, hi) column ranges covering [0, total)."""
    step = (total + n - 1) // n
    return [(j, min(j + step, total)) for j in range(0, total, step)]


def _strips(n, h):
    return STRIPS_LAST if (n == BPC - 1 and h == OH - 1) else STRIPS_STD


def _build_bass():
    f32 = mybir.dt.float32
    AOP = mybir.AluOpType
    nc = bacc.Bacc("TRN2", target_bir_lowering=False, debug=False,
                   num_devices=N_CORES)
    x_d = nc.dram_tensor("x", [BPC, NPLANES, CIN, PLANE], MMDT,
                         kind="ExternalInput").ap()
    # transformed weights: [i, (oh, j, kh, o128)]
    k_d = nc.dram_tensor("k", [CIN, KCOLS], MMDT, kind="ExternalInput").ap()
    b_d = nc.dram_tensor("b", [OH, 128, 1], f32, kind="ExternalInput").ap()
    # output in per-strip block layout: (strip, phase k, row, quad), fp16
    o_d = nc.dram_tensor("o", [BPC, OH, 128, NPIX], MMDT,
                         kind="ExternalOutput").ap()

    _rr = [0]

    def dma(engines, dst, src):
        eng = engines[_rr[0] % len(engines)]
        _rr[0] += 1
        eng.dma_start(dst, src)

    with tile.TileContext(nc) as tc:
        with tc.tile_pool(name="xp", bufs=1) as xp, \
             tc.tile_pool(name="wp", bufs=1) as wpool, \
             tc.tile_pool(name="kp", bufs=1) as kp, \
             tc.tile_pool(name="bp", bufs=1) as bp, \
             tc.tile_pool(name="wu", bufs=1) as wu, \
             tc.tile_pool(name="tp", bufs=8) as tp, \
             tc.tile_pool(name="ps", bufs=8, space="PSUM") as ps, \
             tc.tile_pool(name="ev", bufs=24) as ev, \
             tc.tile_pool(name="at", bufs=8) as at, \
             tc.tile_pool(name="op", bufs=3) as op:

            kt = kp.tile([CIN, KCOLS], MMDT, tag="k")
            bt = bp.tile([128, OH], f32, tag="bias")
            # 6 phase planes per image: plane p holds cols 4q+p (q=0..15)
            xts = [[xp.tile([CIN, PLANE], MMDT, tag=f"x{n}p{p}",
                            name=f"x{n}p{p}") for p in range(NPLANES)]
                   for n in range(BPC)]
            # transformed planes: [i, (j, row, quad)]
            wts = [wpool.tile([CIN, NJ * PLANE], MMDT, tag=f"w{n}",
                              name=f"w{n}") for n in range(BPC)]

            wt = None
            if WARMUP:
                # fp16 so each dummy matmul is a single cheap pass
                wt = wu.tile([128, 128], MMDT, tag="warm")
                nc.vector.memset(wt[:], 0.0)

            # --- input DMAs: explicit per-ring priority waves. The three
            # rings (sync/scalar HWDGE, gpsimd SWDGE) each drain FIFO at
            # ~1/3 of HBM bw, so the critical path (planes p0/p2/p4 +
            # (oh0,j0) weights -> w0 -> first matmul) leads each ring. ---
            S, G, C = nc.sync, nc.gpsimd, nc.scalar
            B0C = RB[0][1] * NQ          # cols of transform block 0: 608
            KJ = KH_EFF * 128            # kt cols per (oh, j): 896

            def ktj(eng, j0, j1):
                eng.dma_start(kt[:, j0 * KJ:j1 * KJ], k_d[:, j0 * KJ:j1 * KJ])

            def plane(eng, n, p, c0, c1):
                eng.dma_start(xts[n][p][:, c0:c1], x_d[n, p][:, c0:c1])

            K3 = [(0, 300), (300, 600), (600, KJ)]   # (oh0, j0) 3-way split
            # wave 1: w0's planes + j0 weights
            plane(S, 0, 0, 0, B0C)
            plane(C, 0, 2, 0, B0C)
            plane(G, 0, 4, 0, B0C)
            for eng, (lo, hi) in zip((S, G, C), K3):
                eng.dma_start(kt[:, lo:hi], k_d[:, lo:hi])
            # wave 2: w1/w2's planes, j1/j2 weights
            plane(S, 0, 1, 0, B0C)
            plane(C, 0, 3, 0, B0C)
            ktj(G, 1, 2)
            ktj(C, 2, 3)
            # wave 3: j3..j5 weights, w5's plane, bias
            ktj(S, 3, 4)
            plane(S, 0, 5, 0, B0C)
            ktj(C, 4, 5)
            ktj(G, 5, 6)
            for h in range(OH):
                C.dma_start(bt[:, h:h + 1], b_d[h])
            # wave 4: img0 remaining rows of all planes
            for p, eng in zip(range(NPLANES), (S, G, C, S, G, C)):
                plane(eng, 0, p, B0C, PLANE)
            # wave 5: img1 planes (DVE queue reaches img1 transforms early)
            for p, eng in zip(range(NPLANES), (S, G, C, S, G, C)):
                plane(eng, 1, p, 0, PLANE)
            # wave 6: oh1 weights
            for eng, (lo, hi) in zip((S, G, C), _splits(KCOLS // 2, 3)):
                off = KCOLS // 2
                eng.dma_start(kt[:, off + lo:off + hi],
                              k_d[:, off + lo:off + hi])

            # --- HAM warmup: dummy matmuls while inputs stream in ---
            for _ in range(WARMUP):
                pw = ps.tile([128, 512], f32, tag="acc")
                nc.tensor.matmul(pw[:, :128], wt[:], wt[:], start=True,
                                 stop=True)

            wvs = [wts[n][:].rearrange("p (j r q) -> p j r q", j=NJ, r=HP)
                   for n in range(BPC)]

            def transform(n, r0, r1):
                """F(4,3) input transform for rows [r0, r1): 6 points from
                planes d0..d5; all reads/writes contiguous fp16 (DVE 2x)."""
                c0, c1 = r0 * NQ, r1 * NQ
                blk = c1 - c0
                d = [xts[n][p][:, c0:c1] for p in range(NPLANES)]
                wv = wvs[n]

                def w(j):
                    return wv[:, j, r0:r1, :]

                cnt = [0]

                def tmp():
                    cnt[0] += 1
                    return tp.tile([CIN, RB[0][1] * NQ], MMDT, tag="t",
                                   name=f"t_{n}_{r0}_{cnt[0]}")

                # j-order: w_j is produced right before the PE's j-group
                # consumes it, so the first matmul starts after just 3 ops
                V = nc.vector
                a = tmp(); V.tensor_sub(a[:, :blk], d[0], d[2])
                b = tmp(); V.tensor_sub(b[:, :blk], d[2], d[4])
                V.scalar_tensor_tensor(w(0), a[:, :blk], 4.0, b[:, :blk],
                                       AOP.mult, AOP.subtract)
                p_ = tmp(); V.tensor_add(p_[:, :blk], d[1], d[2])
                q_ = tmp(); V.tensor_add(q_[:, :blk], d[3], d[4])
                V.scalar_tensor_tensor(w(1), p_[:, :blk], -4.0, q_[:, :blk],
                                       AOP.mult, AOP.add)
                r_ = tmp(); V.tensor_sub(r_[:, :blk], d[1], d[2])
                s_ = tmp(); V.tensor_sub(s_[:, :blk], d[3], d[4])
                V.scalar_tensor_tensor(w(2), r_[:, :blk], 4.0, s_[:, :blk],
                                       AOP.mult, AOP.subtract)
                e_ = tmp(); V.tensor_sub(e_[:, :blk], d[3], d[1])
                f_ = tmp(); V.tensor_sub(f_[:, :blk], d[4], d[2])
                V.scalar_tensor_tensor(w(3), e_[:, :blk], 2.0, f_[:, :blk],
                                       AOP.mult, AOP.add)
                V.scalar_tensor_tensor(w(4), e_[:, :blk], -2.0, f_[:, :blk],
                                       AOP.mult, AOP.add)
                g_ = tmp(); V.tensor_sub(g_[:, :blk], d[3], d[5])
                V.scalar_tensor_tensor(w(5), e_[:, :blk], -4.0, g_[:, :blk],
                                       AOP.mult, AOP.subtract)

            def do_strip(n, h, y0, y1, last):
                rows = y1 - y0
                ncol = rows * NQ
                wv = wvs[n]
                # 6 points, each 7 height taps accumulated in one PSUM bank
                ms = []
                for j in range(NJ):
                    pt = ps.tile([128, 512], mybir.dt.float32, tag="acc",
                                 name=f"m_{n}_{h}_{y0}_{j}")
                    for kh in range(KH_EFF):
                        rhs = wv[:, j, y0 + kh:y0 + kh + rows, :]
                        off = ((h * NJ + j) * KH_EFF + kh) * 128
                        nc.tensor.matmul(pt[:, :ncol], kt[:, off:off + 128],
                                         rhs, start=(kh == 0),
                                         stop=(kh == KH_EFF - 1))
                    mj = ev.tile([128, 512], MMDT, tag="ev",
                                 name=f"ms_{n}_{h}_{y0}_{j}")
                    if j == 1:
                        # A^T's m1 column is all-ones: bias rides here
                        nc.scalar.activation(
                            mj[:, :ncol], pt[:, :ncol],
                            mybir.ActivationFunctionType.Identity,
                            bias=bt[:, h:h + 1])
                    else:
                        nc.scalar.activation(
                            mj[:, :ncol], pt[:, :ncol],
                            mybir.ActivationFunctionType.Copy)
                    ms.append(mj[:, :ncol])

                # output transform on DVE (all fp16 contiguous, 2x mode)
                # y0=m0+m1+m2+m3+m4; y1=m1-m2+2(m3-m4);
                # y2=m1+m2+4(m3+m4); y3=m1-m2+8(m3-m4)+m5
                cnt = [0]

                def tmp():
                    cnt[0] += 1
                    return at.tile([128, 512], MMDT, tag="a",
                                   name=f"a_{n}_{h}_{y0}_{cnt[0]}")

                V = nc.vector
                ot = op.tile([128, 4 * 512], MMDT, tag="out",
                             name=f"ot_{n}_{h}_{y0}")
                ov = ot[:, :4 * ncol].rearrange("p (k c) -> p k c", k=4)
                s1 = tmp(); V.tensor_add(s1[:, :ncol], ms[1], ms[2])
                d1 = tmp(); V.tensor_sub(d1[:, :ncol], ms[1], ms[2])
                t1 = tmp(); V.tensor_add(t1[:, :ncol], ms[3], ms[4])
                u1 = tmp(); V.tensor_sub(u1[:, :ncol], ms[3], ms[4])
                v_ = tmp(); V.tensor_add(v_[:, :ncol], ms[0], s1[:, :ncol])
                V.tensor_add(ov[:, 0, :], v_[:, :ncol], t1[:, :ncol])
                V.scalar_tensor_tensor(ov[:, 1, :], u1[:, :ncol], 2.0,
                                       d1[:, :ncol], AOP.mult, AOP.add)
                V.scalar_tensor_tensor(ov[:, 2, :], t1[:, :ncol], 4.0,
                                       s1[:, :ncol], AOP.mult, AOP.add)
                w_ = tmp(); V.tensor_add(w_[:, :ncol], d1[:, :ncol], ms[5])
                V.scalar_tensor_tensor(ov[:, 3, :], u1[:, :ncol], 8.0,
                                       w_[:, :ncol], AOP.mult, AOP.add)

                # store: (strip rows) * 64 output cols, block layout
                base = y0 * W
                tot = 4 * ncol
                if last:
                    oe = [nc.sync, nc.scalar]
                    for lo, hi in _splits(tot, 2):
                        dma(oe, o_d[n, h][:, base + lo:base + hi],
                            ot[:, lo:hi])
                else:
                    dma([nc.sync, nc.gpsimd], o_d[n, h][:, base:base + tot],
                        ot[:, :tot])

            # transforms for both images lead the DVE queue (they only
            # depend on input DMAs); assemblies follow with their deps
            for n in range(BPC):
                for r0, r1 in (RB0 if n == 0 else RB):
                    transform(n, r0, r1)
            for n in range(BPC):
                for h in range(OH):
                    ss = _strips(n, h)
                    for si, (y0, y1) in enumerate(ss):
                        is_last = (n == BPC - 1 and h == OH - 1
                                   and si == len(ss) - 1)
                        do_strip(n, h, y0, y1, is_last)
    t0 = time.time()
    nc.compile()
    print(f"[kernel] bacc compile: {time.time()-t0:.1f}s", file=sys.stderr)
    return nc


def _host_prep(x, weight, bias, P):
    K = _build_kernel_np(weight, P[0])                    # (O, I, 7, 3)
    g = K.reshape(OH, 128, CIN, KH_EFF, K_W)
    gw = np.einsum('jw,moikw->mjoik', G_F43, g)           # (OH,NJ,o,i,kh)
    k_dev = np.ascontiguousarray(
        gw.transpose(3, 0, 1, 4, 2).reshape(CIN, KCOLS)).astype(NPDT)

    xpad = np.zeros((B, CIN, HP, WP), np.float32)
    xpad[:, :, PAD_H:PAD_H + H, PAD_W:PAD_W + W] = x
    xpad = xpad.astype(NPDT)
    xpl = np.empty((B, NPLANES, CIN, HP, NQ), NPDT)
    for k in range(NPLANES):
        xpl[:, k] = xpad[:, :, :, k::4][:, :, :, :NQ]
    xpl = xpl.reshape(B, NPLANES, CIN, PLANE)

    b_dev = np.ascontiguousarray(bias.reshape(OH, 128, 1)).astype(np.float32)
    return xpl, k_dev, b_dev


def _unpack_core(o_core):
    """(BPC, OH, 128, 4096) fp16 block layout -> (BPC, 256, 64, 64) f32."""
    out = np.empty((BPC, OH, 128, H, W), np.float32)
    for n in range(BPC):
        for h in range(OH):
            for (y0, y1) in _strips(n, h):
                rows = y1 - y0
                blk = o_core[n, h][:, y0 * W:y1 * W].reshape(
                    128, 4, rows, NQ).astype(np.float32)
                # [c, k, r, q] -> [c, r, q, k] -> (c, rows, 64)
                out[n, h, :, y0:y1, :] = blk.transpose(0, 2, 3, 1).reshape(
                    128, rows, W)
    return out.reshape(BPC, COUT, H, W)


def kernel(x: np.ndarray, weight: np.ndarray, bias: np.ndarray,
           P: np.ndarray) -> np.ndarray:
    global _NC_CACHE, _last_in_maps
    x = np.asarray(x, dtype=np.float32)
    weight = np.asarray(weight, dtype=np.float32)
    bias = np.asarray(bias, dtype=np.float32)
    P = np.asarray(P, dtype=np.float32)

    xpl, k_dev, b_dev = _host_prep(x, weight, bias, P)

    if _NC_CACHE is None:
        t0 = time.time()
        _NC_CACHE = _build_bass()
        print(f"[kernel] build+compile total: {time.time()-t0:.1f}s",
              file=sys.stderr)

    in_maps = [
        {"x": np.ascontiguousarray(xpl[i * BPC:(i + 1) * BPC]),
         "k": k_dev, "b": b_dev}
        for i in range(N_CORES)
    ]
    _last_in_maps = in_maps
    t0 = time.time()
    last_exc = None
    for attempt in range(3):
        try:
            res = bass_utils.run_bass_kernel_spmd(
                _NC_CACHE, in_maps, core_ids=list(range(N_CORES)))
            break
        except Exception as e:  # transient device hiccup: retry
            last_exc = e
            print(f"[kernel] run attempt {attempt} failed: {e!r}; retrying",
                  file=sys.stderr)
            time.sleep(5)
    else:
        raise last_exc
    print(f"[kernel] run (incl. walrus compile on first call): "
          f"{time.time()-t0:.1f}s", file=sys.stderr)
    out = np.concatenate(
        [_unpack_core(res.results[i]["o"].reshape(BPC, OH, 128, NPIX))
         for i in range(N_CORES)], axis=0)
    return out


# revision 21
# speedup vs baseline: 1.0144x; 1.0144x over previous
"""Dcls2_1d (dilated conv with learnable row spacings) on 8 trn2 NeuronCores.

Strategy: data-parallel over batch (16 -> 2 images/core). Host constructs the
dense (O, I, 7, 3) scattered kernel (exact port of the reference bilinear
scatter) and F(4,3)-transforms it over the width taps; each core runs the conv
as an implicit GEMM contracting over C_in=128 (the partition dim).

Winograd F(4,3) over width: 6 multiply-points per 4 output columns instead of
12, cutting the PE's streamed matmul columns to half of the direct method
(172k cols/core -> ~72us streaming floor at 1 col/cycle/2.4GHz). The 7 height
taps stay direct, accumulated in PSUM per 32-row strip (512-col PSUM banks).

DVE throughput hygiene (TRN2 tensor_tensor only reaches 2x mode for 16-bit
step-1 4B-aligned operands):
 - host pre-splits the padded image into 6 width-phase planes
   (cols 4q+k, k=0..5) so every input-transform read is contiguous fp16;
 - the output transform writes 4 per-phase blocks (contiguous) instead of
   interleaving; the host de-interleaves after the run;
 - m-points are evacuated PSUM->SBUF as fp16 by the scalar engine (bias for
   the whole A^T rides on m1, whose output coefficients are all ones).

Outputs are DMA'd as fp16 (halves store traffic); the host converts to f32.
Measured rel err (max-abs / max|expected|): ~4.5e-3 vs the fp32 reference.

Input DMAs are priority-ordered (first transform block's planes + first tap
weights first), issued from three engine queues in parallel. A burst of
dummy matmuls warms the PE clock gate (HAM) while inputs are in flight.
"""
import os
import sys
import time

sys.path.insert(0, "/opt/trn_rl_repo")

import numpy as np

import concourse.bass as bass
import concourse.tile as tile
from concourse import bacc, mybir
from concourse import bass_utils

# ---- problem constants (hardcoded per contract) ----
K_H, K_W = 3, 3
LIM = 2            # DIL // 2
KH_EFF = 7         # K_H + 2 * LIM
PAD_H, PAD_W = 3, 1
B, CIN, H, W = 16, 128, 64, 64
COUT = 256
N_CORES = 8
BPC = B // N_CORES                  # images per core
HP, WP = H + 2 * PAD_H, W + 2 * PAD_W   # 70, 66
NPIX = H * W                        # 4096
OH = COUT // 128                    # 2 halves of out channels

NJ = 6                              # F(4,3) points
NQ = W // 4                         # output quads per row: 16
NPLANES = 6                         # width-phase input planes
PLANE = HP * NQ                     # cols per plane: 1120
KCOLS = OH * NJ * KH_EFF * 128      # 10752
RB = [(0, 38), (38, 70)]            # input-transform row blocks (img1)
RB0 = [(0, 38), (38, 64), (64, 70)]  # img0: split so strip 1 unblocks early
STRIPS_STD = [(0, 32), (32, 64)]
STRIPS_LAST = [(0, 32), (32, 48), (48, 64)]   # finer tail on the last block

WARMUP = int(os.environ.get("DCLS_WARMUP", "40"))

MMDT = mybir.dt.float16
NPDT = np.float16

_NC_CACHE = None
_last_in_maps = None  # stashed for test.py's profiled re-run

# F(4,3) weight transform (correlation convention, points 0,+-1,+-2,inf)
G_F43 = np.array([
    [1 / 4, 0, 0],
    [-1 / 6, -1 / 6, -1 / 6],
    [-1 / 6, 1 / 6, -1 / 6],
    [1 / 24, 1 / 12, 1 / 6],
    [1 / 24, -1 / 12, 1 / 6],
    [0, 0, 1],
], np.float32)


def _build_kernel_np(weight: np.ndarray, P1: np.ndarray) -> np.ndarray:
    """Exact numpy port of reference.build_kernel (fp32)."""
    weight = weight.astype(np.float32, copy=False)
    kh = np.arange(K_H, dtype=np.float32)[None, None, :, None]
    pos = kh + LIM + np.clip(P1.astype(np.float32, copy=False), -LIM, LIM)
    p0 = np.floor(pos)
    frac = pos - p0
    p0i = p0.astype(np.int32)
    rng = np.arange(KH_EFF, dtype=np.int32)
    oh0 = (p0i[..., None] == rng).astype(np.float32)
    oh1 = ((p0i + 1)[..., None] == rng).astype(np.float32)
    return (
        np.einsum("oihw,oihwk->oikw", weight * (1.0 - frac), oh0)
        + np.einsum("oihw,oihwk->oikw", weight * frac, oh1)
    ).astype(np.float32)


def _splits(total, n):
    """n near-equal [lo, hi) column ranges covering [0, total)."""
    step = (total + n - 1) // n
    return [(j, min(j + step, total)) for j in range(0, total, step)]


def _strips(n, h):
    return STRIPS_LAST if (n == BPC - 1 and h == OH - 1) else STRIPS_STD


def _build_bass():
    f32 = mybir.dt.float32
    AOP = mybir.AluOpType
    nc = bacc.Bacc("TRN2", target_bir_lowering=False, debug=False,
                   num_devices=N_CORES)
    x_d = nc.dram_tensor("x", [BPC, NPLANES, CIN, PLANE], MMDT,
                         kind="ExternalInput").ap()
    # transformed weights: [i, (oh, j, kh, o128)]
    k_d = nc.dram_tensor("k", [CIN, KCOLS], MMDT, kind="ExternalInput").ap()
    b_d = nc.dram_tensor("b", [OH, 128, 1], f32, kind="ExternalInput").ap()
    # output in per-strip block layout: (strip, phase k, row, quad), fp16
    o_d = nc.dram_tensor("o", [BPC, OH, 128, NPIX], MMDT,
                         kind="ExternalOutput").ap()

    _rr = [0]

    def dma(engines, dst, src):
        eng = engines[_rr[0] % len(engines)]
        _rr[0] += 1
        eng.dma_start(dst, src)

    with tile.TileContext(nc) as tc:
        with tc.tile_pool(name="xp", bufs=1) as xp, \
             tc.tile_pool(name="wp", bufs=1) as wpool, \
             tc.tile_pool(name="kp", bufs=1) as kp, \
             tc.tile_pool(name="bp", bufs=1) as bp, \
             tc.tile_pool(name="wu", bufs=1) as wu, \
             tc.tile_pool(name="tp", bufs=8) as tp, \
             tc.tile_pool(name="ps", bufs=8, space="PSUM") as ps, \
             tc.tile_pool(name="ev", bufs=24) as ev, \
             tc.tile_pool(name="at", bufs=8) as at, \
             tc.tile_pool(name="op", bufs=3) as op:

            kt = kp.tile([CIN, KCOLS], MMDT, tag="k")
            bt = bp.tile([128, OH], f32, tag="bias")
            # 6 phase planes per image: plane p holds cols 4q+p (q=0..15)
            xts = [[xp.tile([CIN, PLANE], MMDT, tag=f"x{n}p{p}",
                            name=f"x{n}p{p}") for p in range(NPLANES)]
                   for n in range(BPC)]
            # transformed planes: [i, (j, row, quad)]
            wts = [wpool.tile([CIN, NJ * PLANE], MMDT, tag=f"w{n}",
                              name=f"w{n}") for n in range(BPC)]

            wt = None
            if WARMUP:
                # fp16 so each dummy matmul is a single cheap pass
                wt = wu.tile([128, 128], MMDT, tag="warm")
                nc.vector.memset(wt[:], 0.0)

            # --- input DMAs: explicit per-ring priority waves. The three
            # rings (sync/scalar HWDGE, gpsimd SWDGE) each drain FIFO at
            # ~1/3 of HBM bw, so the critical path (planes p0/p2/p4 +
            # (oh0,j0) weights -> w0 -> first matmul) leads each ring. ---
            S, G, C = nc.sync, nc.gpsimd, nc.scalar
            B0C = RB[0][1] * NQ          # cols of transform block 0: 608
            KJ = KH_EFF * 128            # kt cols per (oh, j): 896

            def ktj(eng, j0, j1):
                eng.dma_start(kt[:, j0 * KJ:j1 * KJ], k_d[:, j0 * KJ:j1 * KJ])

            def plane(eng, n, p, c0, c1):
                eng.dma_start(xts[n][p][:, c0:c1], x_d[n, p][:, c0:c1])

            def kthalf(j, lo_eng, hi_eng):
                mid = j * KJ + KJ // 2
                lo_eng.dma_start(kt[:, j * KJ:mid], k_d[:, j * KJ:mid])
                hi_eng.dma_start(kt[:, mid:(j + 1) * KJ],
                                 k_d[:, mid:(j + 1) * KJ])

            # wave 1: w0's planes + j0 weights (gpsimd SWDGE is the slow
            # ring: it gets only planes, never schedule-critical weights)
            plane(S, 0, 0, 0, B0C)
            plane(C, 0, 2, 0, B0C)
            plane(G, 0, 4, 0, B0C)
            kthalf(0, S, C)
            # wave 2: w1/w2's planes, j1 weights, bias
            plane(S, 0, 1, 0, B0C)
            plane(C, 0, 3, 0, B0C)
            plane(G, 0, 5, 0, B0C)
            kthalf(1, S, C)
            for h in range(OH):
                C.dma_start(bt[:, h:h + 1], b_d[h])
            # wave 3: j2..j5 weights; img0 remaining plane rows on gpsimd
            ktj(S, 2, 3)
            ktj(C, 3, 4)
            plane(G, 0, 4, B0C, PLANE)
            ktj(S, 4, 5)
            ktj(C, 5, 6)
            for p, eng in ((0, S), (2, C), (1, G), (3, G), (5, G)):
                plane(eng, 0, p, B0C, PLANE)
            # wave 4: img1 planes (DVE queue reaches img1 transforms early)
            for p, eng in zip(range(NPLANES), (S, C, G, S, C, G)):
                plane(eng, 1, p, 0, PLANE)
            # wave 5: oh1 weights
            for eng, (lo, hi) in zip((S, C, G), _splits(KCOLS // 2, 3)):
                off = KCOLS // 2
                eng.dma_start(kt[:, off + lo:off + hi],
                              k_d[:, off + lo:off + hi])

            # --- HAM warmup: dummy matmuls while inputs stream in ---
            for _ in range(WARMUP):
                pw = ps.tile([128, 512], f32, tag="acc")
                nc.tensor.matmul(pw[:, :128], wt[:], wt[:], start=True,
                                 stop=True)

            wvs = [wts[n][:].rearrange("p (j r q) -> p j r q", j=NJ, r=HP)
                   for n in range(BPC)]

            tcnt = [0]

            def transform(n, blocks):
                """F(4,3) input transform for the row blocks [(r0,r1),...].
                Emitted j-stage-major across blocks so each point w_j is
                fully produced in the order the PE consumes it; within the
                first stage the first block leads so matmuls start after
                just 3 ops. All reads/writes contiguous fp16 (DVE 2x)."""
                wv = wvs[n]
                V = nc.vector

                def tmp():
                    tcnt[0] += 1
                    return tp.tile([CIN, RB[0][1] * NQ], MMDT, tag="t",
                                   name=f"t_{tcnt[0]}")

                # per j-stage: (temp specs, w ops); d_k indices + ops
                def stage(mk):
                    for r0, r1 in blocks:
                        c0, c1 = r0 * NQ, r1 * NQ
                        blk = c1 - c0
                        d = [xts[n][p][:, c0:c1] for p in range(NPLANES)]
                        mk(blk, d,
                           lambda j, r0=r0, r1=r1: wv[:, j, r0:r1, :])

                def j0(blk, d, w):
                    a = tmp(); V.tensor_sub(a[:, :blk], d[0], d[2])
                    b = tmp(); V.tensor_sub(b[:, :blk], d[2], d[4])
                    V.scalar_tensor_tensor(w(0), a[:, :blk], 4.0, b[:, :blk],
                                           AOP.mult, AOP.subtract)

                def j1(blk, d, w):
                    p_ = tmp(); V.tensor_add(p_[:, :blk], d[1], d[2])
                    q_ = tmp(); V.tensor_add(q_[:, :blk], d[3], d[4])
                    V.scalar_tensor_tensor(w(1), p_[:, :blk], -4.0,
                                           q_[:, :blk], AOP.mult, AOP.add)

                def j2(blk, d, w):
                    r_ = tmp(); V.tensor_sub(r_[:, :blk], d[1], d[2])
                    s_ = tmp(); V.tensor_sub(s_[:, :blk], d[3], d[4])
                    V.scalar_tensor_tensor(w(2), r_[:, :blk], 4.0,
                                           s_[:, :blk], AOP.mult,
                                           AOP.subtract)

                def j34(blk, d, w):
                    e_ = tmp(); V.tensor_sub(e_[:, :blk], d[3], d[1])
                    f_ = tmp(); V.tensor_sub(f_[:, :blk], d[4], d[2])
                    V.scalar_tensor_tensor(w(3), e_[:, :blk], 2.0,
                                           f_[:, :blk], AOP.mult, AOP.add)
                    V.scalar_tensor_tensor(w(4), e_[:, :blk], -2.0,
                                           f_[:, :blk], AOP.mult, AOP.add)
                    g_ = tmp(); V.tensor_sub(g_[:, :blk], d[3], d[5])
                    V.scalar_tensor_tensor(w(5), e_[:, :blk], -4.0,
                                           g_[:, :blk], AOP.mult,
                                           AOP.subtract)

                for mk in (j0, j1, j2, j34):
                    stage(mk)

            def do_strip(n, h, y0, y1, last):
                rows = y1 - y0
                ncol = rows * NQ
                wv = wvs[n]
                # 6 points, each 7 height taps accumulated in one PSUM bank
                ms = []
                for j in range(NJ):
                    pt = ps.tile([128, 512], mybir.dt.float32, tag="acc",
                                 name=f"m_{n}_{h}_{y0}_{j}")
                    for kh in range(KH_EFF):
                        rhs = wv[:, j, y0 + kh:y0 + kh + rows, :]
                        off = ((h * NJ + j) * KH_EFF + kh) * 128
                        nc.tensor.matmul(pt[:, :ncol], kt[:, off:off + 128],
                                         rhs, start=(kh == 0),
                                         stop=(kh == KH_EFF - 1))
                    mj = ev.tile([128, 512], MMDT, tag="ev",
                                 name=f"ms_{n}_{h}_{y0}_{j}")
                    if j == 1:
                        # A^T's m1 column is all-ones: bias rides here
                        nc.scalar.activation(
                            mj[:, :ncol], pt[:, :ncol],
                            mybir.ActivationFunctionType.Identity,
                            bias=bt[:, h:h + 1])
                    else:
                        nc.scalar.activation(
                            mj[:, :ncol], pt[:, :ncol],
                            mybir.ActivationFunctionType.Copy)
                    ms.append(mj[:, :ncol])

                # output transform on DVE (all fp16 contiguous, 2x mode)
                # y0=m0+m1+m2+m3+m4; y1=m1-m2+2(m3-m4);
                # y2=m1+m2+4(m3+m4); y3=m1-m2+8(m3-m4)+m5
                cnt = [0]

                def tmp():
                    cnt[0] += 1
                    return at.tile([128, 512], MMDT, tag="a",
                                   name=f"a_{n}_{h}_{y0}_{cnt[0]}")

                V = nc.vector
                ot = op.tile([128, 4 * 512], MMDT, tag="out",
                             name=f"ot_{n}_{h}_{y0}")
                ov = ot[:, :4 * ncol].rearrange("p (k c) -> p k c", k=4)
                s1 = tmp(); V.tensor_add(s1[:, :ncol], ms[1], ms[2])
                d1 = tmp(); V.tensor_sub(d1[:, :ncol], ms[1], ms[2])
                t1 = tmp(); V.tensor_add(t1[:, :ncol], ms[3], ms[4])
                u1 = tmp(); V.tensor_sub(u1[:, :ncol], ms[3], ms[4])
                v_ = tmp(); V.tensor_add(v_[:, :ncol], ms[0], s1[:, :ncol])
                V.tensor_add(ov[:, 0, :], v_[:, :ncol], t1[:, :ncol])
                V.scalar_tensor_tensor(ov[:, 1, :], u1[:, :ncol], 2.0,
                                       d1[:, :ncol], AOP.mult, AOP.add)
                V.scalar_tensor_tensor(ov[:, 2, :], t1[:, :ncol], 4.0,
                                       s1[:, :ncol], AOP.mult, AOP.add)
                w_ = tmp(); V.tensor_add(w_[:, :ncol], d1[:, :ncol], ms[5])
                V.scalar_tensor_tensor(ov[:, 3, :], u1[:, :ncol], 8.0,
                                       w_[:, :ncol], AOP.mult, AOP.add)

                # store: (strip rows) * 64 output cols, block layout
                base = y0 * W
                tot = 4 * ncol
                if last:
                    oe = [nc.sync, nc.scalar]
                    for lo, hi in _splits(tot, 2):
                        dma(oe, o_d[n, h][:, base + lo:base + hi],
                            ot[:, lo:hi])
                else:
                    dma([nc.sync, nc.gpsimd], o_d[n, h][:, base:base + tot],
                        ot[:, :tot])

            # transforms for both images lead the DVE queue (they only
            # depend on input DMAs); assemblies follow with their deps
            transform(0, [RB0[0]])
            transform(0, RB0[1:])
            for r0, r1 in RB:
                transform(1, [(r0, r1)])
            for n in range(BPC):
                for h in range(OH):
                    ss = _strips(n, h)
                    for si, (y0, y1) in enumerate(ss):
                        is_last = (n == BPC - 1 and h == OH - 1
                                   and si == len(ss) - 1)
                        do_strip(n, h, y0, y1, is_last)
    t0 = time.time()
    nc.compile()
    print(f"[kernel] bacc compile: {time.time()-t0:.1f}s", file=sys.stderr)
    return nc


def _host_prep(x, weight, bias, P):
    K = _build_kernel_np(weight, P[0])                    # (O, I, 7, 3)
    g = K.reshape(OH, 128, CIN, KH_EFF, K_W)
    gw = np.einsum('jw,moikw->mjoik', G_F43, g)           # (OH,NJ,o,i,kh)
    k_dev = np.ascontiguousarray(
        gw.transpose(3, 0, 1, 4, 2).reshape(CIN, KCOLS)).astype(NPDT)

    xpad = np.zeros((B, CIN, HP, WP), np.float32)
    xpad[:, :, PAD_H:PAD_H + H, PAD_W:PAD_W + W] = x
    xpad = xpad.astype(NPDT)
    xpl = np.empty((B, NPLANES, CIN, HP, NQ), NPDT)
    for k in range(NPLANES):
        xpl[:, k] = xpad[:, :, :, k::4][:, :, :, :NQ]
    xpl = xpl.reshape(B, NPLANES, CIN, PLANE)

    b_dev = np.ascontiguousarray(bias.reshape(OH, 128, 1)).astype(np.float32)
    return xpl, k_dev, b_dev


def _unpack_core(o_core):
    """(BPC, OH, 128, 4096) fp16 block layout -> (BPC, 256, 64, 64) f32."""
    out = np.empty((BPC, OH, 128, H, W), np.float32)
    for n in range(BPC):
        for h in range(OH):
            for (y0, y1) in _strips(n, h):
                rows = y1 - y0
                blk = o_core[n, h][:, y0 * W:y1 * W].reshape(
                    128, 4, rows, NQ).astype(np.float32)
                # [c, k, r, q] -> [c, r, q, k] -> (c, rows, 64)
                out[n, h, :, y0:y1, :] = blk.transpose(0, 2, 3, 1).reshape(
                    128, rows, W)
    return out.reshape(BPC, COUT, H, W)


def kernel(x: np.ndarray, weight: np.ndarray, bias: np.ndarray,
           P: np.ndarray) -> np.ndarray:
    global _NC_CACHE, _last_in_maps
    x = np.asarray(x, dtype=np.float32)
    weight = np.asarray(weight, dtype=np.float32)
    bias = np.asarray(bias, dtype=np.float32)
    P = np.asarray(P, dtype=np.float32)

    xpl, k_dev, b_dev = _host_prep(x, weight, bias, P)

    if _NC_CACHE is None:
        t0 = time.time()
        _NC_CACHE = _build_bass()
        print(f"[kernel] build+compile total: {time.time()-t0:.1f}s",
              file=sys.stderr)

    in_maps = [
        {"x": np.ascontiguousarray(xpl[i * BPC:(i + 1) * BPC]),
         "k": k_dev, "b": b_dev}
        for i in range(N_CORES)
    ]
    _last_in_maps = in_maps
    t0 = time.time()
    last_exc = None
    for attempt in range(3):
        try:
            res = bass_utils.run_bass_kernel_spmd(
                _NC_CACHE, in_maps, core_ids=list(range(N_CORES)))
            break
        except Exception as e:  # transient device hiccup: retry
            last_exc = e
            print(f"[kernel] run attempt {attempt} failed: {e!r}; retrying",
                  file=sys.stderr)
            time.sleep(5)
    else:
        raise last_exc
    print(f"[kernel] run (incl. walrus compile on first call): "
          f"{time.time()-t0:.1f}s", file=sys.stderr)
    out = np.concatenate(
        [_unpack_core(res.results[i]["o"].reshape(BPC, OH, 128, NPIX))
         for i in range(N_CORES)], axis=0)
    return out
